# revision 24
# baseline (speedup 1.0000x reference)
"""MipRayMarcher2 (EG3D volume rendering composite) on 8 Trainium2 NeuronCores.

Math (reference, f32):
    deltas   = depths[:,1:] - depths[:,:-1]
    dmid     = 0.5*(densities[:,:-1] + densities[:,1:])
    sp       = softplus(dmid - 1)
    dd       = sp * deltas
    alpha    = 1 - exp(-dd)
    T        = cumprod([1, 1-alpha+1e-10])[:-1]      (exclusive transmittance)
    weights  = alpha * T
    rgb      = sum(weights * 0.5*(c+c'), s) * 2 - 1
    depth    = sum(weights * 0.5*(d+d'), s) / sum(weights)   (+nan/clip epilogue)

Key identities used on device:
    T_{i+1} = T_i * e_i with e_i = exp(-dd_i)  (the +1e-10 is below f32
    resolution for this data: e_i >~ 0.3, ulp ~ 3e-8 >> 1e-10)
    weights_i   = T_i - T_{i+1}                 (telescoping)
    u_s         = w_{s-1} + w_s = T_{s-1} - T_{s+1}
    rgb_c       = sum_s u_s * c_{s,c} - 1       (0.5 and *2 cancel)
    depth       = 0.5 * sum_s u_s * d_s / (1 - T_95)
    sum weights = T_0 - T_95 = 1 - T_95

Layout: rays on partitions, samples along the free axis.  K rays are packed
per partition per tile (segment-strided access patterns).  The cumprod runs
as ONE flat `tensor_tensor_scan` (op0=mult, op1=max) over all K segments:
each 98-wide segment is [pre, reset, e_0..e_94, hold] with data0=0/data1=1 at
pre+reset (forces state:=1, so ray-to-ray chaining is harmless) and
data0=1/data1=0 at hold (state carries through -> duplicates T_95).
The scan output segment is exactly [T_-1=1, T_0=1, T_1..T_95, T_95], from
which w and u are single shifted subtractions.

Sharding: embarrassingly parallel over rays.  B*R = 65536 rays are split
into 8 contiguous chunks of 8192; each core runs the identical program.
"""

import os
import sys
from contextlib import ExitStack, nullcontext

import numpy as np

for _p in ("/opt/trn_rl_repo", "/root/.axon_site/_ro/trn_rl_repo"):
    if os.path.isdir(_p) and _p not in sys.path:
        sys.path.insert(0, _p)

import concourse.bass as bass  # noqa: E402,F401
import concourse.tile as tile  # noqa: E402
from concourse import bacc, mybir  # noqa: E402
from concourse.bass_utils import run_bass_kernel_spmd  # noqa: E402

B, R, S, C = 4, 16384, 96, 3
NCORES = 8
RAYS = B * R                 # 65536 total rays
RPC = RAYS // NCORES         # 8192 rays per core
P = 128                      # SBUF partitions
SM = S - 1                   # 95 midpoint samples
SEG = S + 2                  # 98: [pre, reset, e*95, hold]
F32 = mybir.dt.float32
Alu = mybir.AluOpType
Act = mybir.ActivationFunctionType
AxisX = mybir.AxisListType.X

DEFAULT_ENG = {
    "dmid2": "gpsimd",   # densities pair-add
    "deltas": "gpsimd",  # depths shifted sub
    "dd": "vector",      # sp * deltas
    "w": "vector",       # A shifted sub (weights)
    "u": "vector",       # A shifted sub (w_{s-1}+w_s)
    "wc3": "gpsimd",     # colors * u broadcast
    "wd": "vector",      # depths * u
    "sd": "scalar",      # sum(wd): "scalar" = ACT per-seg accum, else DVE
    "stage": "all",      # "all" | "dma" (DMA-only skeleton for roofline)
    # probe flags (bench-only; break output correctness where noted):
    "skip_rgb": False,     # drop wc3+rgb reduce+rgb out (colors still loaded)
    "skip_depth": False,   # drop wd/sd/sw/div/depth out
    "skip_w": False,       # drop weights sub + weights out
    "scan_as_copy": False, # replace the scan with a plain tensor_copy
    "scan": "vector",      # engine for the transmittance scan
    "cumsum": "scan",      # "scan" (DVE recurrence) | "pe" (matmul w/ tri L)
    "ddtcopy": "scalar",   # engine for ddT PSUM->SBUF copies in pe mode
}


def host_consts(eng: dict | None = None) -> dict:
    """Extra ExternalInput arrays (identical on every core) for pe mode."""
    e = dict(DEFAULT_ENG)
    if eng:
        e.update(eng)
    if e.get("cumsum") != "pe":
        return {}
    # L[s, j]: A-array slot j gets exp(-sum_{s' <= i} dd) for j = 2+i;
    # slots 0,1 -> 0 (exp -> 1), slot 97 -> full sum (T_95 hold)
    L = np.zeros((SM, SEG), np.float32)
    for i in range(SM):
        L[: i + 1, 2 + i] = 1.0
    L[:, SEG - 1] = 1.0
    ident = np.eye(P, dtype=np.float32)
    return {"Lconst": L, "Ident": ident}


def build_nc(n_rays: int = RPC, k: int = 8, bufs: int = 3, reps: int = 1,
             eng: dict | None = None):
    """Build the per-core Bass program for `n_rays` rays (tiles of 128*k).

    reps > 1 wraps the whole (idempotent) computation in a dynamic For_i
    loop; used only by the benchmark harness to measure per-rep HW time."""
    e = dict(DEFAULT_ENG)
    if eng:
        e.update(eng)
    tr = P * k              # rays per tile
    nt = n_rays // tr       # tiles per core
    assert n_rays % tr == 0

    nc = bacc.Bacc("TRN2", target_bir_lowering=False, debug=False,
                   num_devices=NCORES)

    col_d = nc.dram_tensor("colors", [n_rays, S * C], F32,
                           kind="ExternalInput").ap()
    den_d = nc.dram_tensor("densities", [n_rays, S], F32,
                           kind="ExternalInput").ap()
    dep_d = nc.dram_tensor("depths", [n_rays, S], F32,
                           kind="ExternalInput").ap()
    w_d = nc.dram_tensor("weights", [n_rays, SM], F32,
                         kind="ExternalOutput").ap()
    rgb_d = nc.dram_tensor("rgb", [n_rays, C], F32,
                           kind="ExternalOutput").ap()
    dpt_d = nc.dram_tensor("depth", [n_rays, 1], F32,
                           kind="ExternalOutput").ap()

    # DRAM tile views: ray index = t*tr + p*k + j  (row-major within a tile)
    col_v = col_d.rearrange("(t p k) m -> t p (k m)", t=nt, p=P, k=k)
    den_v = den_d.rearrange("(t p k) m -> t p (k m)", t=nt, p=P, k=k)
    dep_v = dep_d.rearrange("(t p k) m -> t p (k m)", t=nt, p=P, k=k)
    wo_v = w_d.rearrange("(t p k) m -> t p (k m)", t=nt, p=P, k=k)
    rgbo_v = rgb_d.rearrange("(t p k) m -> t p (k m)", t=nt, p=P, k=k)
    dpto_v = dpt_d.rearrange("(t p k) m -> t p (k m)", t=nt, p=P, k=k)

    pe_mode = e["cumsum"] == "pe"
    if pe_mode:
        lc_d = nc.dram_tensor("Lconst", [SM, SEG], F32,
                              kind="ExternalInput").ap()
        id_d = nc.dram_tensor("Ident", [P, P], F32,
                              kind="ExternalInput").ap()

    with tile.TileContext(nc) as tc, ExitStack() as ctx:
        consts = ctx.enter_context(tc.tile_pool(name="consts", bufs=1))
        io = ctx.enter_context(tc.tile_pool(name="io", bufs=bufs))
        mid = ctx.enter_context(tc.tile_pool(name="mid", bufs=bufs))
        sml = ctx.enter_context(tc.tile_pool(name="sml", bufs=bufs))
        if pe_mode:
            psum = ctx.enter_context(
                tc.tile_pool(name="psum", bufs=2, space="PSUM"))
            lct = consts.tile([SM, SEG], F32)
            idt = consts.tile([P, P], F32)
            nc.sync.dma_start(lct[:], lc_d[:])
            nc.sync.dma_start(idt[:], id_d[:])

        # scan data1 constant: 1.0 at [pre, reset], 0.0 elsewhere.
        d1t = consts.tile([P, k * SEG], F32)
        nc.gpsimd.memset(d1t[:], 0.0)
        nc.gpsimd.memset(
            d1t[:].rearrange("p (k g) -> p k g", g=SEG)[:, :, 0:2], 1.0)
        # per-partition bias constant for softplus(0.5*x - 1)
        bm1 = consts.tile([P, 1], F32)
        nc.gpsimd.memset(bm1[:], -1.0)


        zw = zr = zd = None
        if e["stage"] == "dma":
            zw = consts.tile([P, k * SM], F32)
            zr = consts.tile([P, k * C], F32)
            zd = consts.tile([P, k], F32)
            nc.gpsimd.memset(zw[:], 0.0)
            nc.gpsimd.memset(zr[:], 0.0)
            nc.gpsimd.memset(zd[:], 0.0)

        ge = lambda name: getattr(nc, e[name])  # noqa: E731

        def body():
            for t in range(nt):
                ct = io.tile([P, k * S * C], F32, tag="ct")
                dnt = io.tile([P, k * S], F32, tag="dnt")
                dpt = io.tile([P, k * S], F32, tag="dpt")
                nc.sync.dma_start(ct[:], col_v[t])
                nc.sync.dma_start(dnt[:], den_v[t])
                nc.sync.dma_start(dpt[:], dep_v[t])

                if e["stage"] == "dma":
                    # DMA skeleton only: measures the memory roofline
                    nc.sync.dma_start(wo_v[t], zw[:])
                    nc.sync.dma_start(rgbo_v[t], zr[:])
                    nc.sync.dma_start(dpto_v[t], zd[:])
                    continue

                dn3 = dnt[:].rearrange("p (k s) -> p k s", s=S)
                dp3 = dpt[:].rearrange("p (k s) -> p k s", s=S)

                # densities_mid*2 then softplus(0.5*x - 1)
                mt = mid.tile([P, k * S], F32, tag="mt")
                m3 = mt[:].rearrange("p (k s) -> p k s", s=S)
                ge("dmid2").tensor_add(m3[:, :, 0:SM], dn3[:, :, 0:SM],
                                       dn3[:, :, 1:S])
                # softplus(0.5*m - 1) = ln(1 + exp(0.5*m - 1)); exp+ln share
                # one ACT table (no Softplus table exists on TRN2).
                # Both run in place on mt to save SBUF (more pipeline bufs).
                nc.scalar.activation(m3[:, :, 0:SM], m3[:, :, 0:SM],
                                     Act.Exp, scale=0.5, bias=bm1[:])
                nc.scalar.activation(m3[:, :, 0:SM], m3[:, :, 0:SM],
                                     Act.Ln, scale=1.0, bias=1.0)

                # deltas
                dlt = mid.tile([P, k * S], F32, tag="dlt")
                dl3 = dlt[:].rearrange("p (k s) -> p k s", s=S)
                ge("deltas").tensor_sub(dl3[:, :, 0:SM], dp3[:, :, 1:S],
                                        dp3[:, :, 0:SM])

                if pe_mode:
                    # density_delta = sp * deltas -> own tile
                    ddt = mid.tile([P, k * S], F32, tag="ddt")
                    dd3 = ddt[:].rearrange("p (k s) -> p k s", s=S)
                    ge("dd").tensor_mul(dd3[:, :, 0:SM], m3[:, :, 0:SM],
                                        dl3[:, :, 0:SM])
                    # cumsum via PE: per segment transpose dd -> ddT, then
                    # csum[r, :] = sum_s ddT[s, r] * L[s, :]
                    ddts = mid.tile([P, k * P], F32, tag="ddts")
                    csum = psum.tile([P, k * P], F32, tag="csum")
                    for j in range(k):
                        ddtp = psum.tile([P, P], F32, tag="ddtp")
                        nc.tensor.transpose(ddtp[0:SM, :],
                                            dd3[:, j, 0:SM], idt[:])
                        if e["ddtcopy"] == "scalar":
                            nc.scalar.copy(ddts[0:SM, j * P:j * P + P],
                                           ddtp[0:SM, :])
                        else:
                            nc.vector.tensor_copy(
                                ddts[0:SM, j * P:j * P + P], ddtp[0:SM, :])
                        nc.tensor.matmul(
                            csum[:, j * P:j * P + SEG],
                            ddts[0:SM, j * P:j * P + P],
                            lct[0:SM, :])
                    # A = exp(-csum) straight out of PSUM
                    at = mid.tile([P, k * SEG], F32, tag="at")
                    nc.scalar.activation(
                        at[:].rearrange("p (k g) -> p k g", g=SEG),
                        csum[:].rearrange("p (k g) -> p k g", g=P)
                        [:, :, 0:SEG],
                        Act.Exp, scale=-1.0)
                else:
                    # density_delta = sp * deltas, written straight into the
                    # scan-source segment slots [2:97]; then e = exp(-dd) in
                    # place.  Const slots: [0:2]=0.0 (reset), [97]=1.0 (hold)
                    esc = mid.tile([P, k * SEG], F32, tag="esc")
                    e3 = esc[:].rearrange("p (k g) -> p k g", g=SEG)
                    nc.gpsimd.memset(e3[:, :, 0:2], 0.0)
                    nc.gpsimd.memset(e3[:, :, SEG - 1:SEG], 1.0)
                    ge("dd").tensor_mul(e3[:, :, 2:SEG - 1], m3[:, :, 0:SM],
                                        dl3[:, :, 0:SM])
                    nc.scalar.activation(e3[:, :, 2:SEG - 1],
                                         e3[:, :, 2:SEG - 1],
                                         Act.Exp, scale=-1.0)

                    # transmittance via flat scan: state = (d0*state) max d1
                    at = mid.tile([P, k * SEG], F32, tag="at")
                    if e["scan_as_copy"]:
                        nc.vector.tensor_copy(at[:], esc[:])
                    else:
                        ge("scan").tensor_tensor_scan(at[:], esc[:], d1t[:],
                                                      0.0, op0=Alu.mult,
                                                      op1=Alu.max)
                a3 = at[:].rearrange("p (k g) -> p k g", g=SEG)

                # weights = A[1:96] - A[2:97] -> packed [P, k*95] staging
                if not e["skip_w"]:
                    wst = mid.tile([P, k * SM], F32, tag="wst")
                    w3 = wst[:].rearrange("p (k s) -> p k s", s=SM)
                    ge("w").tensor_sub(w3, a3[:, :, 1:S], a3[:, :, 2:S + 1])
                    nc.sync.dma_start(wo_v[t], wst[:])

                # u_s = w_{s-1}+w_s = A[0:96] - A[2:98]
                ut = mid.tile([P, k * S], F32, tag="ut")
                u3 = ut[:].rearrange("p (k s) -> p k s", s=S)
                ge("u").tensor_sub(u3, a3[:, :, 0:S], a3[:, :, 2:S + 2])

                if not e["skip_rgb"]:
                    # colors *= u (broadcast over channel), in place
                    c4 = ct[:].rearrange("p (k s c) -> p k s c", s=S, c=C)
                    u4 = u3.unsqueeze(3).broadcast_to([P, k, S, C])
                    ge("wc3").tensor_mul(c4, c4, u4)

                    # rgb = sum_s (colors*u) - 1
                    rgba = sml.tile([P, k * C], F32, tag="rgba")
                    nc.vector.reduce_sum(
                        rgba[:].rearrange("p (k c) -> p k c", c=C),
                        ct[:].rearrange("p (k s c) -> p k c s", s=S, c=C),
                        axis=AxisX)
                    rgbo = sml.tile([P, k * C], F32, tag="rgbo")
                    nc.scalar.activation(rgbo[:], rgba[:], Act.Copy,
                                         bias=-1.0)
                    nc.sync.dma_start(rgbo_v[t], rgbo[:])

                if e["skip_depth"]:
                    continue
                # wd = depths * u (in place over depths tile)
                ge("wd").tensor_mul(dp3[:, :, 0:S], dp3[:, :, 0:S], u3)

                # sd_j = 0.5 * sum_s wd[j, s]
                sdt = sml.tile([P, k], F32, tag="sdt")
                if e["sd"] == "scalar":
                    # ACT copy w/ accumulate, per segment; copy output is
                    # discarded - reuse dlt (dead after dd) as the sink
                    for j in range(k):
                        nc.scalar.activation(dl3[:, j, 0:S], dp3[:, j, :],
                                             Act.Copy, scale=0.5,
                                             accum_out=sdt[:, j:j + 1])
                else:
                    nc.vector.tensor_reduce(sdt[:], dp3[:, :, 0:S],
                                            axis=AxisX, op=Alu.add)

                # sum(weights) = 1 - T_95
                swt = sml.tile([P, k], F32, tag="swt")
                nc.scalar.activation(
                    swt[:], a3[:, :, SEG - 1:SEG].squeeze(2), Act.Copy,
                    scale=-1.0, bias=1.0)

                # depth = sd / sw (reciprocal + multiply; divide is not in
                # the DVE scalar_tensor_tensor ISA op set).  With sd on DVE
                # the 0.5 is folded here via scalar_tensor_tensor instead.
                rcpt = sml.tile([P, k], F32, tag="rcpt")
                nc.vector.reciprocal(rcpt[:], swt[:])
                cdt = sml.tile([P, k], F32, tag="cdt")
                if e["sd"] == "scalar":
                    nc.vector.tensor_mul(cdt[:], sdt[:], rcpt[:])
                else:
                    nc.vector.scalar_tensor_tensor(
                        cdt[:], sdt[:], 0.5, rcpt[:],
                        op0=Alu.mult, op1=Alu.mult)
                nc.sync.dma_start(dpto_v[t], cdt[:])

        if reps > 1:
            with tc.For_i(0, reps, 1):
                body()
        else:
            body()

    nc.compile()
    return nc


_NC_CACHE: dict = {}


def _get_nc(n_rays: int, k: int = 8):
    key = (n_rays, k)
    if key not in _NC_CACHE:
        _NC_CACHE[key] = build_nc(n_rays, k)
    return _NC_CACHE[key]


def kernel(colors: np.ndarray, densities: np.ndarray, depths: np.ndarray):
    colors = np.ascontiguousarray(colors, dtype=np.float32)
    densities = np.ascontiguousarray(densities, dtype=np.float32)
    depths = np.ascontiguousarray(depths, dtype=np.float32)

    col2 = colors.reshape(RAYS, S * C)
    den2 = densities.reshape(RAYS, S)
    dep2 = depths.reshape(RAYS, S)

    nc = _get_nc(RPC)
    cm = host_consts()
    in_maps = []
    for c in range(NCORES):
        sl = slice(c * RPC, (c + 1) * RPC)
        in_maps.append({
            "colors": col2[sl],
            "densities": den2[sl],
            "depths": dep2[sl],
            **cm,
        })
    res = run_bass_kernel_spmd(nc, in_maps, core_ids=list(range(NCORES)))

    w = np.concatenate([r["weights"] for r in res.results], axis=0)
    rgb = np.concatenate([r["rgb"] for r in res.results], axis=0)
    dpt = np.concatenate([r["depth"] for r in res.results], axis=0)

    # reference epilogue: nan->inf then clip to global depth range
    dpt = np.clip(np.nan_to_num(dpt, nan=np.inf, posinf=np.inf),
                  depths.min(), depths.max())

    composite_rgb = rgb.reshape(B, R, C)
    composite_depth = dpt.reshape(B, R, 1)
    weights = w.reshape(B, R, SM, 1)
    return composite_rgb, composite_depth, weights


# revision 28
# speedup vs baseline: 1.1322x; 1.1322x over previous
"""MipRayMarcher2 (EG3D volume rendering composite) on 8 Trainium2 NeuronCores.

Math (reference, f32):
    deltas   = depths[:,1:] - depths[:,:-1]
    dmid     = 0.5*(densities[:,:-1] + densities[:,1:])
    sp       = softplus(dmid - 1)
    dd       = sp * deltas
    alpha    = 1 - exp(-dd)
    T        = cumprod([1, 1-alpha+1e-10])[:-1]      (exclusive transmittance)
    weights  = alpha * T
    rgb      = sum(weights * 0.5*(c+c'), s) * 2 - 1
    depth    = sum(weights * 0.5*(d+d'), s) / sum(weights)   (+nan/clip epilogue)

Key identities used on device:
    T_{i+1} = T_i * e_i with e_i = exp(-dd_i)  (the +1e-10 is below f32
    resolution for this data: e_i >~ 0.3, ulp ~ 3e-8 >> 1e-10)
    weights_i   = T_i - T_{i+1}                 (telescoping)
    u_s         = w_{s-1} + w_s = T_{s-1} - T_{s+1}
    rgb_c       = sum_s u_s * c_{s,c} - 1       (0.5 and *2 cancel)
    depth       = 0.5 * sum_s u_s * d_s / (1 - T_95)
    sum weights = T_0 - T_95 = 1 - T_95

Layout: rays on partitions, samples along the free axis.  K rays are packed
per partition per tile (segment-strided access patterns).  The cumprod runs
as ONE flat `tensor_tensor_scan` (op0=mult, op1=max) over all K segments:
each 98-wide segment is [pre, reset, e_0..e_94, hold] with data0=0/data1=1 at
pre+reset (forces state:=1, so ray-to-ray chaining is harmless) and
data0=1/data1=0 at hold (state carries through -> duplicates T_95).
The scan output segment is exactly [T_-1=1, T_0=1, T_1..T_95, T_95], from
which w and u are single shifted subtractions.

Sharding: embarrassingly parallel over rays.  B*R = 65536 rays are split
into 8 contiguous chunks of 8192; each core runs the identical program.
"""

import os
import sys
from contextlib import ExitStack, nullcontext

import numpy as np

for _p in ("/opt/trn_rl_repo", "/root/.axon_site/_ro/trn_rl_repo"):
    if os.path.isdir(_p) and _p not in sys.path:
        sys.path.insert(0, _p)

import concourse.bass as bass  # noqa: E402,F401
import concourse.tile as tile  # noqa: E402
from concourse import bacc, mybir  # noqa: E402
from concourse.bass_utils import run_bass_kernel_spmd  # noqa: E402

# The act-table-load pass greedily picks the FIRST table containing each
# activation function: Exp -> "exp_and_others" (id 0), Ln -> "natural_log"
# (id 5).  An Exp/Ln/Exp sequence then reloads the ACT table twice per tile
# (~17 InstLoadActFuncSet per kernel).  Restrict Exp and Ln to the one table
# that has BOTH ("natural_log_exp_and_others") so exactly one load is
# emitted.  Names/indices are preserved, so the act_func_set_id the pass
# writes stays a valid index into act_info.json.
from concourse.hw_specs import get_activation_tables as _gat  # noqa: E402


def _gat_single_table(arch):
    tabs = _gat(arch)
    for name, funcs in tabs.items():
        if name != "natural_log_exp_and_others":
            funcs.discard(mybir.ActivationFunctionType.Exp)
            funcs.discard(mybir.ActivationFunctionType.Ln)
    return tabs


bacc.get_activation_tables = _gat_single_table

B, R, S, C = 4, 16384, 96, 3
NCORES = 8
RAYS = B * R                 # 65536 total rays
RPC = RAYS // NCORES         # 8192 rays per core
P = 128                      # SBUF partitions
SM = S - 1                   # 95 midpoint samples
SEG = S + 2                  # 98: [pre, reset, e*95, hold]
F32 = mybir.dt.float32
Alu = mybir.AluOpType
Act = mybir.ActivationFunctionType
AxisX = mybir.AxisListType.X

DEFAULT_ENG = {
    "dmid2": "gpsimd",   # densities pair-add
    "deltas": "gpsimd",  # depths shifted sub
    "dd": "vector",      # sp * deltas
    "w": "vector",       # A shifted sub (weights)
    "u": "vector",       # A shifted sub (w_{s-1}+w_s)
    "wc3": "gpsimd",     # colors * u broadcast
    "wd": "vector",      # depths * u
    "sd": "scalar",      # sum(wd): "scalar" = ACT per-seg accum, else DVE
    "stage": "all",      # "all" | "dma" (DMA-only skeleton for roofline)
    # probe flags (bench-only; break output correctness where noted):
    "skip_rgb": False,     # drop wc3+rgb reduce+rgb out (colors still loaded)
    "skip_depth": False,   # drop wd/sd/sw/div/depth out
    "skip_w": False,       # drop weights sub + weights out
    "scan_as_copy": False, # replace the scan with a plain tensor_copy
    "scan": "vector",      # engine for the transmittance scan
    "cumsum": "scan",      # "scan" (DVE recurrence) | "pe" (matmul w/ tri L)
    "ddtcopy": "scalar",   # engine for ddT PSUM->SBUF copies in pe mode
    "rgb_m1_host": False,  # apply the rgb "-1" on the host after gather
}


def host_consts(eng: dict | None = None) -> dict:
    """Extra ExternalInput arrays (identical on every core) for pe mode."""
    e = dict(DEFAULT_ENG)
    if eng:
        e.update(eng)
    if e.get("cumsum") != "pe":
        return {}
    # L[s, j]: A-array slot j gets exp(-sum_{s' <= i} dd) for j = 2+i;
    # slots 0,1 -> 0 (exp -> 1), slot 97 -> full sum (T_95 hold)
    L = np.zeros((SM, SEG), np.float32)
    for i in range(SM):
        L[: i + 1, 2 + i] = 1.0
    L[:, SEG - 1] = 1.0
    ident = np.eye(P, dtype=np.float32)
    return {"Lconst": L, "Ident": ident}


def build_nc(n_rays: int = RPC, k: int = 8, bufs: int = 3, reps: int = 1,
             eng: dict | None = None):
    """Build the per-core Bass program for `n_rays` rays (tiles of 128*k).

    reps > 1 wraps the whole (idempotent) computation in a dynamic For_i
    loop; used only by the benchmark harness to measure per-rep HW time."""
    e = dict(DEFAULT_ENG)
    if eng:
        e.update(eng)
    tr = P * k              # rays per tile
    nt = n_rays // tr       # tiles per core
    assert n_rays % tr == 0

    nc = bacc.Bacc("TRN2", target_bir_lowering=False, debug=False,
                   num_devices=NCORES)

    col_d = nc.dram_tensor("colors", [n_rays, S * C], F32,
                           kind="ExternalInput").ap()
    den_d = nc.dram_tensor("densities", [n_rays, S], F32,
                           kind="ExternalInput").ap()
    dep_d = nc.dram_tensor("depths", [n_rays, S], F32,
                           kind="ExternalInput").ap()
    w_d = nc.dram_tensor("weights", [n_rays, SM], F32,
                         kind="ExternalOutput").ap()
    rgb_d = nc.dram_tensor("rgb", [n_rays, C], F32,
                           kind="ExternalOutput").ap()
    dpt_d = nc.dram_tensor("depth", [n_rays, 1], F32,
                           kind="ExternalOutput").ap()

    # DRAM tile views: ray index = t*tr + p*k + j  (row-major within a tile)
    col_v = col_d.rearrange("(t p k) m -> t p (k m)", t=nt, p=P, k=k)
    den_v = den_d.rearrange("(t p k) m -> t p (k m)", t=nt, p=P, k=k)
    dep_v = dep_d.rearrange("(t p k) m -> t p (k m)", t=nt, p=P, k=k)
    wo_v = w_d.rearrange("(t p k) m -> t p (k m)", t=nt, p=P, k=k)
    rgbo_v = rgb_d.rearrange("(t p k) m -> t p (k m)", t=nt, p=P, k=k)
    dpto_v = dpt_d.rearrange("(t p k) m -> t p (k m)", t=nt, p=P, k=k)

    pe_mode = e["cumsum"] == "pe"
    if pe_mode:
        lc_d = nc.dram_tensor("Lconst", [SM, SEG], F32,
                              kind="ExternalInput").ap()
        id_d = nc.dram_tensor("Ident", [P, P], F32,
                              kind="ExternalInput").ap()

    with tile.TileContext(nc) as tc, ExitStack() as ctx:
        consts = ctx.enter_context(tc.tile_pool(name="consts", bufs=1))
        io = ctx.enter_context(tc.tile_pool(name="io", bufs=bufs))
        mid = ctx.enter_context(tc.tile_pool(name="mid", bufs=bufs))
        sml = ctx.enter_context(tc.tile_pool(name="sml", bufs=bufs))
        if pe_mode:
            psum = ctx.enter_context(
                tc.tile_pool(name="psum", bufs=4, space="PSUM"))
            lct = consts.tile([SM, SEG], F32)
            idt = consts.tile([P, P], F32)
            nc.sync.dma_start(lct[:], lc_d[:])
            nc.sync.dma_start(idt[:], id_d[:])

        # scan data1 constant: 1.0 at [pre, reset], 0.0 elsewhere.
        d1t = consts.tile([P, k * SEG], F32)
        nc.gpsimd.memset(d1t[:], 0.0)
        nc.gpsimd.memset(
            d1t[:].rearrange("p (k g) -> p k g", g=SEG)[:, :, 0:2], 1.0)
        # per-partition bias constant for softplus(0.5*x - 1)
        bm1 = consts.tile([P, 1], F32)
        nc.gpsimd.memset(bm1[:], -1.0)


        zw = zr = zd = None
        if e["stage"] == "dma":
            zw = consts.tile([P, k * SM], F32)
            zr = consts.tile([P, k * C], F32)
            zd = consts.tile([P, k], F32)
            nc.gpsimd.memset(zw[:], 0.0)
            nc.gpsimd.memset(zr[:], 0.0)
            nc.gpsimd.memset(zd[:], 0.0)

        ge = lambda name: getattr(nc, e[name])  # noqa: E731

        def body():
            for t in range(nt):
                ct = io.tile([P, k * S * C], F32, tag="ct")
                dnt = io.tile([P, k * S], F32, tag="dnt")
                dpt = io.tile([P, k * S], F32, tag="dpt")
                nc.sync.dma_start(ct[:], col_v[t])
                nc.sync.dma_start(dnt[:], den_v[t])
                nc.sync.dma_start(dpt[:], dep_v[t])

                if e["stage"] == "dma":
                    # DMA skeleton only: measures the memory roofline
                    nc.sync.dma_start(wo_v[t], zw[:])
                    nc.sync.dma_start(rgbo_v[t], zr[:])
                    nc.sync.dma_start(dpto_v[t], zd[:])
                    continue

                dn3 = dnt[:].rearrange("p (k s) -> p k s", s=S)
                dp3 = dpt[:].rearrange("p (k s) -> p k s", s=S)

                # densities_mid*2 then softplus(0.5*x - 1)
                mt = mid.tile([P, k * S], F32, tag="mt")
                m3 = mt[:].rearrange("p (k s) -> p k s", s=S)
                ge("dmid2").tensor_add(m3[:, :, 0:SM], dn3[:, :, 0:SM],
                                       dn3[:, :, 1:S])
                # softplus(0.5*m - 1) = ln(1 + exp(0.5*m - 1)); exp+ln share
                # one ACT table (no Softplus table exists on TRN2).
                # Both run in place on mt to save SBUF (more pipeline bufs).
                nc.scalar.activation(m3[:, :, 0:SM], m3[:, :, 0:SM],
                                     Act.Exp, scale=0.5, bias=bm1[:])
                nc.scalar.activation(m3[:, :, 0:SM], m3[:, :, 0:SM],
                                     Act.Ln, scale=1.0, bias=1.0)

                # deltas
                dlt = mid.tile([P, k * S], F32, tag="dlt")
                dl3 = dlt[:].rearrange("p (k s) -> p k s", s=S)
                ge("deltas").tensor_sub(dl3[:, :, 0:SM], dp3[:, :, 1:S],
                                        dp3[:, :, 0:SM])

                if pe_mode:
                    # density_delta = sp * deltas -> own tile
                    ddt = mid.tile([P, k * S], F32, tag="ddt")
                    dd3 = ddt[:].rearrange("p (k s) -> p k s", s=S)
                    ge("dd").tensor_mul(dd3[:, :, 0:SM], m3[:, :, 0:SM],
                                        dl3[:, :, 0:SM])
                    # cumsum via PE: burst-transpose all segments into one
                    # PSUM tile, ONE wide PSUM->SBUF copy, then burst-matmul
                    # csum[r, :] = sum_s ddT[s, r] * L[s, :]
                    ddtp = psum.tile([P, k * P], F32, tag="ddtp")
                    for j in range(k):
                        nc.tensor.transpose(ddtp[0:SM, j * P:j * P + P],
                                            dd3[:, j, 0:SM], idt[:])
                    ddts = mid.tile([P, k * P], F32, tag="ddts")
                    if e["ddtcopy"] == "scalar":
                        nc.scalar.copy(ddts[0:SM, :], ddtp[0:SM, :])
                    else:
                        nc.vector.tensor_copy(ddts[0:SM, :], ddtp[0:SM, :])
                    # csum reuses the same PSUM tile (ddT is dead after the
                    # copy) so the pool can run twice as deep
                    csum = ddtp
                    for j in range(k):
                        nc.tensor.matmul(
                            csum[:, j * P:j * P + SEG],
                            ddts[0:SM, j * P:j * P + P],
                            lct[0:SM, :])
                    # A = exp(-csum) straight out of PSUM
                    at = mid.tile([P, k * SEG], F32, tag="at")
                    nc.scalar.activation(
                        at[:].rearrange("p (k g) -> p k g", g=SEG),
                        csum[:].rearrange("p (k g) -> p k g", g=P)
                        [:, :, 0:SEG],
                        Act.Exp, scale=-1.0)
                else:
                    # density_delta = sp * deltas, written straight into the
                    # scan-source segment slots [2:97]; then e = exp(-dd) in
                    # place.  Const slots: [0:2]=0.0 (reset), [97]=1.0 (hold)
                    esc = mid.tile([P, k * SEG], F32, tag="esc")
                    e3 = esc[:].rearrange("p (k g) -> p k g", g=SEG)
                    nc.gpsimd.memset(e3[:, :, 0:2], 0.0)
                    nc.gpsimd.memset(e3[:, :, SEG - 1:SEG], 1.0)
                    ge("dd").tensor_mul(e3[:, :, 2:SEG - 1], m3[:, :, 0:SM],
                                        dl3[:, :, 0:SM])
                    nc.scalar.activation(e3[:, :, 2:SEG - 1],
                                         e3[:, :, 2:SEG - 1],
                                         Act.Exp, scale=-1.0)

                    # transmittance via flat scan: state = (d0*state) max d1
                    at = mid.tile([P, k * SEG], F32, tag="at")
                    if e["scan_as_copy"]:
                        nc.vector.tensor_copy(at[:], esc[:])
                    else:
                        ge("scan").tensor_tensor_scan(at[:], esc[:], d1t[:],
                                                      0.0, op0=Alu.mult,
                                                      op1=Alu.max)
                a3 = at[:].rearrange("p (k g) -> p k g", g=SEG)

                # weights = A[1:96] - A[2:97] -> packed [P, k*95] staging
                if not e["skip_w"]:
                    wst = mid.tile([P, k * SM], F32, tag="wst")
                    w3 = wst[:].rearrange("p (k s) -> p k s", s=SM)
                    ge("w").tensor_sub(w3, a3[:, :, 1:S], a3[:, :, 2:S + 1])
                    nc.sync.dma_start(wo_v[t], wst[:])

                # u_s = w_{s-1}+w_s = A[0:96] - A[2:98]; dlt is dead after
                # dd, reuse it as u storage (saves a pool tag -> more bufs)
                u3 = dl3[:, :, 0:S]
                ge("u").tensor_sub(u3, a3[:, :, 0:S], a3[:, :, 2:S + 2])

                if not e["skip_rgb"]:
                    # colors *= u (broadcast over channel), in place
                    c4 = ct[:].rearrange("p (k s c) -> p k s c", s=S, c=C)
                    u4 = u3.unsqueeze(3).broadcast_to([P, k, S, C])
                    ge("wc3").tensor_mul(c4, c4, u4)

                    # rgb = sum_s (colors*u) - 1
                    rgba = sml.tile([P, k * C], F32, tag="rgba")
                    nc.vector.reduce_sum(
                        rgba[:].rearrange("p (k c) -> p k c", c=C),
                        ct[:].rearrange("p (k s c) -> p k c s", s=S, c=C),
                        axis=AxisX)
                    if e["rgb_m1_host"]:
                        nc.sync.dma_start(rgbo_v[t], rgba[:])
                    else:
                        rgbo = sml.tile([P, k * C], F32, tag="rgbo")
                        nc.scalar.activation(rgbo[:], rgba[:], Act.Copy,
                                             bias=-1.0)
                        nc.sync.dma_start(rgbo_v[t], rgbo[:])

                if e["skip_depth"]:
                    continue
                # wd = depths * u (in place over depths tile)
                ge("wd").tensor_mul(dp3[:, :, 0:S], dp3[:, :, 0:S], u3)

                # sd_j = 0.5 * sum_s wd[j, s]
                sdt = sml.tile([P, k], F32, tag="sdt")
                if e["sd"] == "scalar":
                    # ACT copy w/ accumulate, per segment; copy output is
                    # discarded - reuse mt (dead after dd) as the sink
                    for j in range(k):
                        nc.scalar.activation(m3[:, j, 0:S], dp3[:, j, :],
                                             Act.Copy, scale=0.5,
                                             accum_out=sdt[:, j:j + 1])
                else:
                    nc.vector.tensor_reduce(sdt[:], dp3[:, :, 0:S],
                                            axis=AxisX, op=Alu.add)

                # sum(weights) = 1 - T_95
                swt = sml.tile([P, k], F32, tag="swt")
                nc.scalar.activation(
                    swt[:], a3[:, :, SEG - 1:SEG].squeeze(2), Act.Copy,
                    scale=-1.0, bias=1.0)

                # depth = sd / sw (reciprocal + multiply; divide is not in
                # the DVE scalar_tensor_tensor ISA op set).  With sd on DVE
                # the 0.5 is folded here via scalar_tensor_tensor instead.
                rcpt = sml.tile([P, k], F32, tag="rcpt")
                nc.vector.reciprocal(rcpt[:], swt[:])
                cdt = sml.tile([P, k], F32, tag="cdt")
                if e["sd"] == "scalar":
                    nc.vector.tensor_mul(cdt[:], sdt[:], rcpt[:])
                else:
                    nc.vector.scalar_tensor_tensor(
                        cdt[:], sdt[:], 0.5, rcpt[:],
                        op0=Alu.mult, op1=Alu.mult)
                nc.sync.dma_start(dpto_v[t], cdt[:])

        if reps > 1:
            with tc.For_i(0, reps, 1):
                body()
        else:
            body()

    nc.compile()
    return nc


_NC_CACHE: dict = {}


def _get_nc(n_rays: int, k: int = 8):
    key = (n_rays, k)
    if key not in _NC_CACHE:
        _NC_CACHE[key] = build_nc(n_rays, k)
    return _NC_CACHE[key]


def kernel(colors: np.ndarray, densities: np.ndarray, depths: np.ndarray):
    colors = np.ascontiguousarray(colors, dtype=np.float32)
    densities = np.ascontiguousarray(densities, dtype=np.float32)
    depths = np.ascontiguousarray(depths, dtype=np.float32)

    col2 = colors.reshape(RAYS, S * C)
    den2 = densities.reshape(RAYS, S)
    dep2 = depths.reshape(RAYS, S)

    nc = _get_nc(RPC)
    cm = host_consts()
    in_maps = []
    for c in range(NCORES):
        sl = slice(c * RPC, (c + 1) * RPC)
        in_maps.append({
            "colors": col2[sl],
            "densities": den2[sl],
            "depths": dep2[sl],
            **cm,
        })
    res = run_bass_kernel_spmd(nc, in_maps, core_ids=list(range(NCORES)))

    w = np.concatenate([r["weights"] for r in res.results], axis=0)
    rgb = np.concatenate([r["rgb"] for r in res.results], axis=0)
    dpt = np.concatenate([r["depth"] for r in res.results], axis=0)

    # reference epilogue: nan->inf then clip to global depth range
    dpt = np.clip(np.nan_to_num(dpt, nan=np.inf, posinf=np.inf),
                  depths.min(), depths.max())

    composite_rgb = rgb.reshape(B, R, C)
    composite_depth = dpt.reshape(B, R, 1)
    weights = w.reshape(B, R, SM, 1)
    return composite_rgb, composite_depth, weights
